# revision 11
# baseline (speedup 1.0000x reference)
"""Causal multi-head attention on 8 Trainium2 NeuronCores — v3.

Problem: B=4, S=2048, E=2048, H=16 heads (HD=128), fp32 I/O.

Sharding: (batch, head-half).  Core c owns batch b = c//2 and heads
[rank*8, rank*8+8) with rank = c%2, so activations are read once per
batch-pair instead of replicated 8x (v1 read 117MB of HBM per core and
the chip spent 87% of the run power-throttled; v3 reads ~45MB).

Structure (x never resides in SBUF — each activation tensor streams
through exactly once):
  - Phase Q / K: stream xq/xk; for each (span, ec-chunk) one matmul per
    head into 8 parallel psum accumulators (borrowed across all four
    psum pools) -> qT_all / kT_all [P, 8, S] in SBUF.
  - V projection runs directly in [s, d] layout (per-s-chunk psum pair,
    8 heads packed), emitted as fine-grained steps INTERLEAVED between
    the attention chunks' scores and attn@V matmuls: attention alone is
    ACT(exp)-paced (~610ns vs 427ns of PE per chunk), and the PE
    executes its stream in order, so the filler work must sit exactly
    in the would-be stall slots.  Span i only needs v chunks <= 4i+3;
    the steps for chunks 4i+4..4i+7 are threaded through span i.
  - Attention per (head, span): diagonal chunks first with column
    clipping, denominator via ones-vector matmul + fast reciprocal +
    partition broadcast; normalized output kept in SBUF (aoT_all).
  - Out-projection: half-contraction partials (this core's 8 heads,
    straight from SBUF) for ALL 2048 rows against the core's 8
    head-rows of wo; pairwise ReduceScatter ops (valid for 2-core
    groups, unlike AllToAll) sum the pair per 128-row chunk into
    pre-bias bf16 outputs.  Host adds bias and upcasts to fp32.
  - Weights rotate through one 32KB slot (wq -> wk -> wv -> wo), each
    loaded in ec-quarters so the next phase starts after ~1MB arrives.
"""

import numpy as np
import ml_dtypes

import concourse.bacc as bacc
import concourse.mybir as mybir
import concourse.tile as tile
import concourse.bass_utils as bass_utils

B, S, E, H = 4, 2048, 2048, 16
HD = E // H            # 128
N_CORES = 8
H_LOC = 8              # heads per core
F_LOC = H_LOC * HD     # 1024 features per core
S_HALF = S // 2        # 1024 output rows per core
P = 128
NS = 512
EC = E // P            # 16 contraction chunks
QSP = S // NS          # 4 q-spans per head
KCH = S // P           # 16 k-chunks
INV_SQRT_HD = float(1.0 / np.sqrt(HD))

BF16 = mybir.dt.bfloat16
F32 = mybir.dt.float32

_cached_nc = None


def _build():
    nc = bacc.Bacc("TRN2", target_bir_lowering=False, debug=False,
                   num_devices=N_CORES)

    xq_d = nc.dram_tensor("xq", [E, S], BF16, kind="ExternalInput")
    xk_d = nc.dram_tensor("xk", [E, S], BF16, kind="ExternalInput")
    xv_d = nc.dram_tensor("xv", [E, S], BF16, kind="ExternalInput")
    wqt_d = nc.dram_tensor("wqt", [E, F_LOC], BF16, kind="ExternalInput")
    wkt_d = nc.dram_tensor("wkt", [E, F_LOC], BF16, kind="ExternalInput")
    wvt_d = nc.dram_tensor("wvt", [E, F_LOC], BF16, kind="ExternalInput")
    wot_d = nc.dram_tensor("wot", [F_LOC, E], BF16, kind="ExternalInput")
    masks_d = nc.dram_tensor("masks", [4, P, NS], BF16, kind="ExternalInput")
    # 4 output blocks of 256 rows: fewer, larger ReduceScatters (the ops
    # are latency/skew-dominated ~20us regardless of 1-2MB size, and 8 of
    # them paced the out-projection window)
    out_sc_d = [nc.dram_tensor(f"out{j}", [2 * P, E], BF16,
                               kind="ExternalOutput") for j in range(4)]

    xq_v = xq_d.ap().rearrange("(ec p) s -> p ec s", p=P)
    xk_v = xk_d.ap().rearrange("(ec p) s -> p ec s", p=P)
    xv_v = xv_d.ap().rearrange("(ec p) s -> p ec s", p=P)

    with tile.TileContext(nc) as tc:
        with (
            tc.tile_pool(name="wconst", bufs=1) as wconst,
            tc.tile_pool(name="wt", bufs=1) as wtp,
            tc.tile_pool(name="res", bufs=1) as resp,
            tc.tile_pool(name="xs", bufs=3) as xs,
            tc.tile_pool(name="expp", bufs=4) as expp,
            tc.tile_pool(name="smallp", bufs=2) as smallp,
            tc.tile_pool(name="outp", bufs=2) as outp,
            tc.tile_pool(name="ps_sc", bufs=3, space="PSUM") as ps_sc,
            tc.tile_pool(name="ps_acc", bufs=2, space="PSUM") as ps_acc,
            tc.tile_pool(name="ps_den", bufs=1, space="PSUM") as ps_den,
            tc.tile_pool(name="ps_v", bufs=2, space="PSUM") as ps_v,
            tc.tile_pool(name="dram", bufs=1, space="DRAM") as dram,
        ):
            mask_sb = wconst.tile([P, 4, NS], BF16, tag="mask")
            nc.scalar.dma_start(mask_sb[:], masks_d.ap().rearrange("r p q -> p r q"))
            ones_sb = wconst.tile([P, 1], BF16, tag="ones")
            nc.vector.memset(ones_sb[:], 1.0)

            rs_in = [dram.tile([2, 2 * P, E], BF16, tag=f"rs_in{j}",
                               name=f"rs_in{j}") for j in range(4)]
            # collectives may not write IO tensors directly; stage then copy
            rs_out = [dram.tile([2 * P, E], BF16, tag=f"rs_out{j}",
                                name=f"rs_out{j}") for j in range(4)]

            # projected tensors resident for the whole kernel
            qT_all = resp.tile([P, H_LOC, S], BF16, tag="qT")
            kT_all = resp.tile([P, H_LOC, S], BF16, tag="kT")
            v_all = resp.tile([P, KCH, H_LOC, HD], BF16, tag="v")
            aoT_all = resp.tile([P, H_LOC, S], BF16, tag="aoT")

            def load_w(w_ap, name):
                w_sb = wtp.tile([P, EC, F_LOC], BF16, tag="wt", name=name)
                for q in range(4):
                    nc.sync.dma_start(w_sb[:, 4 * q:4 * q + 4, :],
                                      w_ap[:, 4 * q:4 * q + 4, :])
                return w_sb

            def ps8():
                """8 parallel [P,NS] f32 accumulators across the 4 pools."""
                t = [ps_sc.tile([P, NS], F32, tag="sc", name=f"a{z}")
                     for z in range(3)]
                t += [ps_acc.tile([P, NS], F32, tag="acc", name=f"a{3 + z}")
                      for z in range(2)]
                t += [ps_den.tile([P, NS], F32, tag="den", name="a5")]
                t += [ps_v.tile([P, NS], F32, tag="v", name=f"a{6 + z}")
                      for z in range(2)]
                return t

            # ---------------- phase Q / K: T-layout projections ---------
            for w_d, x_v, dst, lbl in ((wqt_d, xq_v, qT_all, "wq"),
                                       (wkt_d, xk_v, kT_all, "wk")):
                w_sb = load_w(w_d.ap().rearrange("(ec p) f -> p ec f", p=P),
                              lbl)
                for sp in range(4):
                    ps = ps8()
                    for g in range(4):
                        x_t = xs.tile([P, 4, NS], BF16, tag="x")
                        nc.sync.dma_start(
                            x_t[:], x_v[:, 4 * g:4 * g + 4,
                                        sp * NS:(sp + 1) * NS])
                        for e4 in range(4):
                            ec = 4 * g + e4
                            for h in range(H_LOC):
                                nc.tensor.matmul(
                                    ps[h][:], w_sb[:, ec, h * HD:(h + 1) * HD],
                                    x_t[:, e4, :],
                                    start=(ec == 0), stop=(ec == EC - 1))
                    for h in range(H_LOC):
                        nc.vector.tensor_copy(dst[:, h, sp * NS:(sp + 1) * NS],
                                              ps[h][:])

            wv_sb = load_w(wvt_d.ap().rearrange("(ec p) f -> p ec f", p=P),
                           "wv")

            def v_chunk_steps(sc):
                """Project s-chunk sc for all heads ([s, d]); 4 yield-steps."""
                ps = [ps_v.tile([P, NS], F32, tag="v", name=f"v{z}")
                      for z in range(2)]
                for g in range(4):
                    x_t = xs.tile([P, 4, P], BF16, tag="xv", bufs=3)
                    nc.sync.dma_start(
                        x_t[:], xv_v[:, 4 * g:4 * g + 4, sc * P:(sc + 1) * P])
                    for e4 in range(4):
                        ec = 4 * g + e4
                        # one matmul per psum bank covering 4 heads: psum
                        # accumulation chains must not interleave within a
                        # bank (only the last-started chain survives)
                        for b4 in range(2):
                            nc.tensor.matmul(
                                ps[b4][:], x_t[:, e4, :],
                                wv_sb[:, ec, b4 * NS:(b4 + 1) * NS],
                                start=(ec == 0), stop=(ec == EC - 1))
                    yield
                for b4 in range(2):
                    nc.vector.tensor_copy(v_all[:, sc, 4 * b4:4 * b4 + 4, :],
                                          ps[b4][:])
                yield

            def v_steps_for(chunks):
                for sc in chunks:
                    yield from v_chunk_steps(sc)

            def drain(gen):
                if gen is not None:
                    for _ in gen:
                        pass

            def attn_span(i, filler):
                """Attention span i for all 8 heads; pulls filler steps
                between scores and attn@V so the PE never waits on exp."""
                n_k = 4 * i + 4
                j_seq = list(range(4 * i, n_k)) + list(range(0, 4 * i))
                n_chunks = H_LOC * n_k
                n_steps = 20  # 4 v-chunks x 5 steps
                stride = max(1, n_chunks // (n_steps + 1))
                t = 0
                for h in range(H_LOC):
                    outT_ps = ps_acc.tile([P, NS], F32, tag="acc")
                    dacc = expp.tile([P, NS], BF16, tag="dacc", bufs=2)
                    for jn, j in enumerate(j_seq):
                        r = j - 4 * i
                        c0 = 128 * r if (0 < r and jn != n_k - 1) else 0
                        s_ps = ps_sc.tile([P, NS], F32, tag="sc")
                        nc.tensor.matmul(
                            s_ps[:, c0:], kT_all[:, h, j * P:(j + 1) * P],
                            qT_all[:, h, i * NS + c0:(i + 1) * NS],
                            start=True, stop=True)
                        e_t = expp.tile([P, NS], BF16, tag="e", bufs=3)
                        nc.scalar.activation(e_t[:, c0:], s_ps[:, c0:],
                                             mybir.ActivationFunctionType.Exp,
                                             scale=INV_SQRT_HD)
                        if r >= 0:
                            nc.vector.tensor_mul(e_t[:, c0:], e_t[:, c0:],
                                                 mask_sb[:, r, c0:])
                        t += 1
                        if filler is not None and t % stride == 0:
                            next(filler, None)
                        if jn == 0:
                            nc.vector.tensor_copy(dacc[:], e_t[:])
                        else:
                            nc.vector.tensor_add(dacc[:, c0:], dacc[:, c0:],
                                                 e_t[:, c0:])
                        nc.tensor.matmul(outT_ps[:, c0:], v_all[:, j, h, :],
                                         e_t[:, c0:],
                                         start=(jn == 0), stop=(jn == n_k - 1))
                    den_ps = ps_den.tile([1, NS], F32, tag="den")
                    nc.tensor.matmul(den_ps[:], ones_sb[:], dacc[:],
                                     start=True, stop=True)
                    aof = smallp.tile([P, NS], BF16, tag="aof", bufs=2)
                    nc.scalar.copy(aof[:], outT_ps[:])
                    den_rec = smallp.tile([1, NS], F32, tag="den_rec", bufs=1)
                    nc.vector.reciprocal_approx_fast(den_rec[:], den_ps[:])
                    den_bc = smallp.tile([P, NS], F32, tag="den_bc", bufs=2)
                    nc.gpsimd.partition_broadcast(den_bc[:], den_rec[:])
                    nc.vector.tensor_mul(aoT_all[:, h, i * NS:(i + 1) * NS],
                                         aof[:], den_bc[:])

            # v chunks 0-3 up front, 4i+4..4i+7 threaded through span i
            drain(v_steps_for(range(0, 4)))
            for i in range(QSP):
                filler = (v_steps_for(range(4 * i + 4, 4 * i + 8))
                          if i < QSP - 1 else None)
                if i == QSP - 1:
                    # wv's slot frees after the last v matmul; wo streams
                    # in during the last attention span (scalar queue).
                    wo_my = wtp.tile([P, H_LOC, E], BF16, tag="wt",
                                     name="wo")
                    for q in range(4):
                        nc.scalar.dma_start(
                            wo_my[:, 2 * q:2 * q + 2, :],
                            wot_d.ap().rearrange("(hc p) f -> p hc f", p=P)
                            [:, 2 * q:2 * q + 2, :])
                attn_span(i, filler)
                drain(filler)

            # ---------------- out-projection + pairwise ReduceScatter ---
            for j in range(4):
                for r in range(2):
                    for s2 in range(2):
                        row0 = r * S_HALF + (2 * j + s2) * P
                        for nf in range(4):
                            psx = ps_sc.tile([P, NS], F32, tag="sc")
                            for hc in range(H_LOC):
                                nc.tensor.matmul(
                                    psx[:], aoT_all[:, hc, row0:row0 + P],
                                    wo_my[:, hc, nf * NS:(nf + 1) * NS],
                                    start=(hc == 0), stop=(hc == H_LOC - 1))
                            p_t = outp.tile([P, NS], BF16, tag="p")
                            nc.vector.tensor_copy(p_t[:], psx[:])
                            nc.scalar.dma_start(
                                rs_in[j][r, s2 * P:(s2 + 1) * P,
                                         nf * NS:(nf + 1) * NS], p_t[:])
                nc.gpsimd.collective_compute(
                    "ReduceScatter", mybir.AluOpType.add,
                    replica_groups=[[0, 1], [2, 3], [4, 5], [6, 7]],
                    ins=[rs_in[j][:].opt()],
                    outs=[rs_out[j][:].opt()])
                if j >= 1:
                    nc.sync.dma_start(out_sc_d[j - 1].ap(), rs_out[j - 1][:])
            nc.sync.dma_start(out_sc_d[3].ap(), rs_out[3][:])

    nc.compile()
    return nc


def _get_nc():
    global _cached_nc
    if _cached_nc is None:
        _cached_nc = _build()
    return _cached_nc


def prep_in_maps(query, key, value, Wq, Wk, Wv, Wo, bo):
    """Host-side layout prep shared by kernel() and the traced test run."""
    bf = ml_dtypes.bfloat16
    qt = np.ascontiguousarray(query.transpose(0, 2, 1)).astype(bf)
    kt = np.ascontiguousarray(key.transpose(0, 2, 1)).astype(bf)
    vt = np.ascontiguousarray(value.transpose(0, 2, 1)).astype(bf)
    wot_f = np.ascontiguousarray(Wo.T.astype(np.float32))

    kk = np.arange(P)[:, None]
    qq = np.arange(NS)[None, :]
    masks = np.stack([(kk <= qq - P * r) for r in range(4)]).astype(bf)

    in_maps = []
    for c in range(N_CORES):
        b, rank = c // 2, c % 2
        sl = slice(rank * F_LOC, (rank + 1) * F_LOC)
        in_maps.append(dict(
            xq=qt[b], xk=kt[b], xv=vt[b],
            wqt=np.ascontiguousarray(Wq[sl].T).astype(bf),
            wkt=np.ascontiguousarray(Wk[sl].T).astype(bf),
            wvt=np.ascontiguousarray(Wv[sl].T).astype(bf),
            wot=np.ascontiguousarray(wot_f[sl]).astype(bf),
            masks=masks,
        ))
    return in_maps


def kernel(query, key, value, key_padding_mask, Wq, Wk, Wv, Wo, bo):
    query = np.asarray(query, dtype=np.float32)
    key = np.asarray(key, dtype=np.float32)
    value = np.asarray(value, dtype=np.float32)
    Wq = np.asarray(Wq, dtype=np.float32)
    Wk = np.asarray(Wk, dtype=np.float32)
    Wv = np.asarray(Wv, dtype=np.float32)
    Wo = np.asarray(Wo, dtype=np.float32)
    bo = np.asarray(bo, dtype=np.float32)

    in_maps = prep_in_maps(query, key, value, Wq, Wk, Wv, Wo, bo)
    nc = _get_nc()
    res = bass_utils.run_bass_kernel_spmd(
        nc, in_maps, core_ids=list(range(N_CORES)), trace=False)

    out = np.empty((B, S, E), dtype=np.float32)
    for c in range(N_CORES):
        b, rank = c // 2, c % 2
        for j in range(4):
            r0 = rank * S_HALF + j * 2 * P
            out[b, r0:r0 + 2 * P, :] = \
                res.results[c][f"out{j}"].astype(np.float32)
    out += bo[None, None, :]
    return out


# revision 13
# speedup vs baseline: 1.1482x; 1.1482x over previous
"""Causal multi-head attention on 8 Trainium2 NeuronCores — v3.

Problem: B=4, S=2048, E=2048, H=16 heads (HD=128), fp32 I/O.

Sharding: (batch, head-half).  Core c owns batch b = c//2 and heads
[rank*8, rank*8+8) with rank = c%2, so activations are read once per
batch-pair instead of replicated 8x (v1 read 117MB of HBM per core and
the chip spent 87% of the run power-throttled; v3 reads ~45MB).

Structure (x never resides in SBUF — each activation tensor streams
through exactly once):
  - Phase Q / K: stream xq/xk; for each (span, ec-chunk) one matmul per
    head into 8 parallel psum accumulators (borrowed across all four
    psum pools) -> qT_all / kT_all [P, 8, S] in SBUF.
  - V projection runs directly in [s, d] layout (per-s-chunk psum pair,
    8 heads packed), emitted as fine-grained steps INTERLEAVED between
    the attention chunks' scores and attn@V matmuls: attention alone is
    ACT(exp)-paced (~610ns vs 427ns of PE per chunk), and the PE
    executes its stream in order, so the filler work must sit exactly
    in the would-be stall slots.  Span i only needs v chunks <= 4i+3;
    the steps for chunks 4i+4..4i+7 are threaded through span i.
  - Attention per (head, span): diagonal chunks first with column
    clipping, denominator via ones-vector matmul + fast reciprocal +
    partition broadcast; normalized output kept in SBUF (aoT_all).
  - Out-projection: half-contraction partials (this core's 8 heads,
    straight from SBUF) for ALL 2048 rows against the core's 8
    head-rows of wo; pairwise ReduceScatter ops (valid for 2-core
    groups, unlike AllToAll) sum the pair per 128-row chunk into
    pre-bias bf16 outputs.  Host adds bias and upcasts to fp32.
  - Weights rotate through one 32KB slot (wq -> wk -> wv -> wo), each
    loaded in ec-quarters so the next phase starts after ~1MB arrives.
"""

import numpy as np
import ml_dtypes

import concourse.bacc as bacc
import concourse.mybir as mybir
import concourse.tile as tile
import concourse.bass_utils as bass_utils

B, S, E, H = 4, 2048, 2048, 16
HD = E // H            # 128
N_CORES = 8
H_LOC = 8              # heads per core
F_LOC = H_LOC * HD     # 1024 features per core
S_HALF = S // 2        # 1024 output rows per core
P = 128
NS = 512
EC = E // P            # 16 contraction chunks
QSP = S // NS          # 4 q-spans per head
KCH = S // P           # 16 k-chunks
INV_SQRT_HD = float(1.0 / np.sqrt(HD))

BF16 = mybir.dt.bfloat16
F32 = mybir.dt.float32

_cached_nc = None


def _build():
    nc = bacc.Bacc("TRN2", target_bir_lowering=False, debug=False,
                   num_devices=N_CORES)

    xq_d = nc.dram_tensor("xq", [E, S], BF16, kind="ExternalInput")
    xk_d = nc.dram_tensor("xk", [E, S], BF16, kind="ExternalInput")
    xv_d = nc.dram_tensor("xv", [E, S], BF16, kind="ExternalInput")
    wqt_d = nc.dram_tensor("wqt", [E, F_LOC], BF16, kind="ExternalInput")
    wkt_d = nc.dram_tensor("wkt", [E, F_LOC], BF16, kind="ExternalInput")
    wvt_d = nc.dram_tensor("wvt", [E, F_LOC], BF16, kind="ExternalInput")
    wot_d = nc.dram_tensor("wot", [F_LOC, E], BF16, kind="ExternalInput")
    masks_d = nc.dram_tensor("masks", [4, P, NS], BF16, kind="ExternalInput")
    # both pair-partials come back raw; the host sums them (and adds bias)
    # - no device collectives at all, so no RS pacing/skew/tail
    outp_d = [nc.dram_tensor(f"out{sc}", [2, P, E], BF16,
                             kind="ExternalOutput")
              for sc in range(S_HALF // P)]

    xq_v = xq_d.ap().rearrange("(ec p) s -> p ec s", p=P)
    xk_v = xk_d.ap().rearrange("(ec p) s -> p ec s", p=P)
    xv_v = xv_d.ap().rearrange("(ec p) s -> p ec s", p=P)

    with tile.TileContext(nc) as tc:
        with (
            tc.tile_pool(name="wconst", bufs=1) as wconst,
            tc.tile_pool(name="wt", bufs=1) as wtp,
            tc.tile_pool(name="res", bufs=1) as resp,
            tc.tile_pool(name="xs", bufs=3) as xs,
            tc.tile_pool(name="expp", bufs=4) as expp,
            tc.tile_pool(name="smallp", bufs=2) as smallp,
            tc.tile_pool(name="outp", bufs=2) as outp,
            tc.tile_pool(name="ps_sc", bufs=3, space="PSUM") as ps_sc,
            tc.tile_pool(name="ps_acc", bufs=2, space="PSUM") as ps_acc,
            tc.tile_pool(name="ps_den", bufs=1, space="PSUM") as ps_den,
            tc.tile_pool(name="ps_v", bufs=2, space="PSUM") as ps_v,
            tc.tile_pool(name="dram", bufs=1, space="DRAM") as dram,
        ):
            mask_sb = wconst.tile([P, 4, NS], BF16, tag="mask")
            nc.scalar.dma_start(mask_sb[:], masks_d.ap().rearrange("r p q -> p r q"))
            ones_sb = wconst.tile([P, 1], BF16, tag="ones")
            nc.vector.memset(ones_sb[:], 1.0)


            # projected tensors resident for the whole kernel
            qT_all = resp.tile([P, H_LOC, S], BF16, tag="qT")
            kT_all = resp.tile([P, H_LOC, S], BF16, tag="kT")
            v_all = resp.tile([P, KCH, H_LOC, HD], BF16, tag="v")
            aoT_all = resp.tile([P, H_LOC, S], BF16, tag="aoT")

            def load_w(w_ap, name):
                w_sb = wtp.tile([P, EC, F_LOC], BF16, tag="wt", name=name)
                for q in range(4):
                    nc.sync.dma_start(w_sb[:, 4 * q:4 * q + 4, :],
                                      w_ap[:, 4 * q:4 * q + 4, :])
                return w_sb

            def ps8():
                """8 parallel [P,NS] f32 accumulators across the 4 pools."""
                t = [ps_sc.tile([P, NS], F32, tag="sc", name=f"a{z}")
                     for z in range(3)]
                t += [ps_acc.tile([P, NS], F32, tag="acc", name=f"a{3 + z}")
                      for z in range(2)]
                t += [ps_den.tile([P, NS], F32, tag="den", name="a5")]
                t += [ps_v.tile([P, NS], F32, tag="v", name=f"a{6 + z}")
                      for z in range(2)]
                return t

            # ---------------- phase Q / K: T-layout projections ---------
            for w_d, x_v, dst, lbl in ((wqt_d, xq_v, qT_all, "wq"),
                                       (wkt_d, xk_v, kT_all, "wk")):
                w_sb = load_w(w_d.ap().rearrange("(ec p) f -> p ec f", p=P),
                              lbl)
                for sp in range(4):
                    ps = ps8()
                    for g in range(4):
                        x_t = xs.tile([P, 4, NS], BF16, tag="x")
                        nc.sync.dma_start(
                            x_t[:], x_v[:, 4 * g:4 * g + 4,
                                        sp * NS:(sp + 1) * NS])
                        for e4 in range(4):
                            ec = 4 * g + e4
                            for h in range(H_LOC):
                                nc.tensor.matmul(
                                    ps[h][:], w_sb[:, ec, h * HD:(h + 1) * HD],
                                    x_t[:, e4, :],
                                    start=(ec == 0), stop=(ec == EC - 1))
                    for h in range(H_LOC):
                        nc.vector.tensor_copy(dst[:, h, sp * NS:(sp + 1) * NS],
                                              ps[h][:])

            wv_sb = load_w(wvt_d.ap().rearrange("(ec p) f -> p ec f", p=P),
                           "wv")

            def v_chunk_steps(sc):
                """Project s-chunk sc for all heads ([s, d]); 4 yield-steps."""
                ps = [ps_v.tile([P, NS], F32, tag="v", name=f"v{z}")
                      for z in range(2)]
                for g in range(4):
                    x_t = xs.tile([P, 4, P], BF16, tag="xv", bufs=3)
                    nc.sync.dma_start(
                        x_t[:], xv_v[:, 4 * g:4 * g + 4, sc * P:(sc + 1) * P])
                    for e4 in range(4):
                        ec = 4 * g + e4
                        # one matmul per psum bank covering 4 heads: psum
                        # accumulation chains must not interleave within a
                        # bank (only the last-started chain survives)
                        for b4 in range(2):
                            nc.tensor.matmul(
                                ps[b4][:], x_t[:, e4, :],
                                wv_sb[:, ec, b4 * NS:(b4 + 1) * NS],
                                start=(ec == 0), stop=(ec == EC - 1))
                    yield
                for b4 in range(2):
                    nc.vector.tensor_copy(v_all[:, sc, 4 * b4:4 * b4 + 4, :],
                                          ps[b4][:])
                yield

            def v_steps_for(chunks):
                for sc in chunks:
                    yield from v_chunk_steps(sc)

            def drain(gen):
                if gen is not None:
                    for _ in gen:
                        pass

            def attn_span(i, filler):
                """Attention span i for all 8 heads; pulls filler steps
                between scores and attn@V so the PE never waits on exp."""
                n_k = 4 * i + 4
                j_seq = list(range(4 * i, n_k)) + list(range(0, 4 * i))
                n_chunks = H_LOC * n_k
                n_steps = 20  # 4 v-chunks x 5 steps
                stride = max(1, n_chunks // (n_steps + 1))
                t = 0
                for h in range(H_LOC):
                    outT_ps = ps_acc.tile([P, NS], F32, tag="acc")
                    dacc = expp.tile([P, NS], BF16, tag="dacc", bufs=2)
                    for jn, j in enumerate(j_seq):
                        r = j - 4 * i
                        c0 = 128 * r if (0 < r and jn != n_k - 1) else 0
                        s_ps = ps_sc.tile([P, NS], F32, tag="sc")
                        nc.tensor.matmul(
                            s_ps[:, c0:], kT_all[:, h, j * P:(j + 1) * P],
                            qT_all[:, h, i * NS + c0:(i + 1) * NS],
                            start=True, stop=True)
                        e_t = expp.tile([P, NS], BF16, tag="e", bufs=3)
                        nc.scalar.activation(e_t[:, c0:], s_ps[:, c0:],
                                             mybir.ActivationFunctionType.Exp,
                                             scale=INV_SQRT_HD)
                        if r >= 0:
                            nc.vector.tensor_mul(e_t[:, c0:], e_t[:, c0:],
                                                 mask_sb[:, r, c0:])
                        t += 1
                        if filler is not None and t % stride == 0:
                            next(filler, None)
                        if jn == 0:
                            nc.vector.tensor_copy(dacc[:], e_t[:])
                        else:
                            nc.vector.tensor_add(dacc[:, c0:], dacc[:, c0:],
                                                 e_t[:, c0:])
                        nc.tensor.matmul(outT_ps[:, c0:], v_all[:, j, h, :],
                                         e_t[:, c0:],
                                         start=(jn == 0), stop=(jn == n_k - 1))
                    den_ps = ps_den.tile([1, NS], F32, tag="den")
                    nc.tensor.matmul(den_ps[:], ones_sb[:], dacc[:],
                                     start=True, stop=True)
                    aof = smallp.tile([P, NS], BF16, tag="aof", bufs=2)
                    nc.scalar.copy(aof[:], outT_ps[:])
                    den_rec = smallp.tile([1, NS], F32, tag="den_rec", bufs=1)
                    nc.vector.reciprocal_approx_fast(den_rec[:], den_ps[:])
                    den_bc = smallp.tile([P, NS], F32, tag="den_bc", bufs=2)
                    nc.gpsimd.partition_broadcast(den_bc[:], den_rec[:])
                    nc.vector.tensor_mul(aoT_all[:, h, i * NS:(i + 1) * NS],
                                         aof[:], den_bc[:])

            # v chunks 0-3 up front, 4i+4..4i+7 threaded through span i
            drain(v_steps_for(range(0, 4)))
            for i in range(QSP):
                filler = (v_steps_for(range(4 * i + 4, 4 * i + 8))
                          if i < QSP - 1 else None)
                if i == QSP - 1:
                    # wv's slot frees after the last v matmul; wo streams
                    # in during the last attention span (scalar queue).
                    wo_my = wtp.tile([P, H_LOC, E], BF16, tag="wt",
                                     name="wo")
                    for q in range(4):
                        nc.scalar.dma_start(
                            wo_my[:, 2 * q:2 * q + 2, :],
                            wot_d.ap().rearrange("(hc p) f -> p hc f", p=P)
                            [:, 2 * q:2 * q + 2, :])
                attn_span(i, filler)
                drain(filler)

            # ---------------- out-projection + pairwise ReduceScatter ---
            n_sc = S_HALF // P
            for sc in range(n_sc):
                for r in range(2):
                    row0 = r * S_HALF + sc * P
                    for nf in range(4):
                        psx = ps_sc.tile([P, NS], F32, tag="sc")
                        for hc in range(H_LOC):
                            nc.tensor.matmul(
                                psx[:], aoT_all[:, hc, row0:row0 + P],
                                wo_my[:, hc, nf * NS:(nf + 1) * NS],
                                start=(hc == 0), stop=(hc == H_LOC - 1))
                        p_t = outp.tile([P, NS], BF16, tag="p")
                        nc.vector.tensor_copy(p_t[:], psx[:])
                        nc.scalar.dma_start(
                            outp_d[sc].ap()[r, :, nf * NS:(nf + 1) * NS],
                            p_t[:])

    nc.compile()
    return nc


def _get_nc():
    global _cached_nc
    if _cached_nc is None:
        _cached_nc = _build()
    return _cached_nc


def prep_in_maps(query, key, value, Wq, Wk, Wv, Wo, bo):
    """Host-side layout prep shared by kernel() and the traced test run."""
    bf = ml_dtypes.bfloat16
    qt = np.ascontiguousarray(query.transpose(0, 2, 1)).astype(bf)
    kt = np.ascontiguousarray(key.transpose(0, 2, 1)).astype(bf)
    vt = np.ascontiguousarray(value.transpose(0, 2, 1)).astype(bf)
    wot_f = np.ascontiguousarray(Wo.T.astype(np.float32))

    kk = np.arange(P)[:, None]
    qq = np.arange(NS)[None, :]
    masks = np.stack([(kk <= qq - P * r) for r in range(4)]).astype(bf)

    in_maps = []
    for c in range(N_CORES):
        b, rank = c // 2, c % 2
        sl = slice(rank * F_LOC, (rank + 1) * F_LOC)
        in_maps.append(dict(
            xq=qt[b], xk=kt[b], xv=vt[b],
            wqt=np.ascontiguousarray(Wq[sl].T).astype(bf),
            wkt=np.ascontiguousarray(Wk[sl].T).astype(bf),
            wvt=np.ascontiguousarray(Wv[sl].T).astype(bf),
            wot=np.ascontiguousarray(wot_f[sl]).astype(bf),
            masks=masks,
        ))
    return in_maps


def kernel(query, key, value, key_padding_mask, Wq, Wk, Wv, Wo, bo):
    query = np.asarray(query, dtype=np.float32)
    key = np.asarray(key, dtype=np.float32)
    value = np.asarray(value, dtype=np.float32)
    Wq = np.asarray(Wq, dtype=np.float32)
    Wk = np.asarray(Wk, dtype=np.float32)
    Wv = np.asarray(Wv, dtype=np.float32)
    Wo = np.asarray(Wo, dtype=np.float32)
    bo = np.asarray(bo, dtype=np.float32)

    in_maps = prep_in_maps(query, key, value, Wq, Wk, Wv, Wo, bo)
    nc = _get_nc()
    res = bass_utils.run_bass_kernel_spmd(
        nc, in_maps, core_ids=list(range(N_CORES)), trace=False)

    out = np.empty((B, S, E), dtype=np.float32)
    for b in range(B):
        ra, rb = res.results[2 * b], res.results[2 * b + 1]
        for sc in range(S_HALF // P):
            for r in range(2):
                r0 = r * S_HALF + sc * P
                out[b, r0:r0 + P, :] = (ra[f"out{sc}"][r].astype(np.float32)
                                        + rb[f"out{sc}"][r].astype(np.float32))
    out += bo[None, None, :]
    return out
